# revision 1
# baseline (speedup 1.0000x reference)
"""Distributed Trainium2 kernel for AlternateWeaveGather (segment_reduce).

Reference computation:
    h = x @ W.T + b                      # [N, 512] linear
    out = segment_mean(h, batch, 256)    # [256, 512]

Since the linear layer commutes with the segment sum:
    out[s] = (segsum_x[s] @ W.T) / max(c[s], 1) + b * (c[s] > 0)

so each core only needs to segment-reduce its row shard of x (one-hot
matmul on the TensorEngine), ReduceScatter the [256, 513] (sums|counts)
across the 8 cores, and run the tiny 512x512 linear on its 32 owned
segments. This turns a 68-GFLOP matmul problem into an HBM-bound
streaming reduction.

Because batch is sorted, each core's 16384 rows span only ~33 contiguous
segment ids, so the one-hot window is 128 wide (one matmul per 128 rows
instead of two 256-wide ones). The host passes batch ids relative to the
core's first segment; an indirect-scatter DMA realigns the local
[128, 513] window into global segment rows before the ReduceScatter.

Sharding: data-parallel over rows. x/batch split along dim 0 across 8
cores; W/b replicated; ReduceScatter combines local sums+counts before
the divide; host concatenates the 8x[32, 512] outputs.
"""

import numpy as np

import concourse.bacc as bacc
import concourse.bass as bass
import concourse.mybir as mybir
import concourse.tile as tile
from concourse.bass_utils import run_bass_kernel_spmd

N_CORES = 8
N_ROWS = 131072
D = 512
N_SEG = 256
SEG_PER_CORE = N_SEG // N_CORES
W_WIN = 128  # one-hot window width (per-core segment span is ~33)

F32 = mybir.dt.float32
F32R = mybir.dt.float32r
I32 = mybir.dt.int32
BF16 = mybir.dt.bfloat16
RS_BF16 = True  # ship the ReduceScatter payload in bf16 (halves transfer)


def build_nc(rows_per_core=N_ROWS // N_CORES):
    assert rows_per_core % 1024 == 0
    n_sup = rows_per_core // 1024  # supertiles of 1024 rows (16KB DMA lines)

    nc = bacc.Bacc("TRN2", target_bir_lowering=False, debug=False,
                   num_devices=N_CORES)
    x = nc.dram_tensor("x", [rows_per_core, D], F32, kind="ExternalInput")
    # batchp[p, k*n_sup + t] = batch_rel[1024t + 8p + k]  (8 row planes)
    batchp = nc.dram_tensor("batchp", [128, 8 * n_sup], F32,
                            kind="ExternalInput")
    idx = nc.dram_tensor("idx", [128, 1], I32, kind="ExternalInput")
    wt = nc.dram_tensor("wt", [D, D], BF16, kind="ExternalInput")
    bb = nc.dram_tensor("bb", [SEG_PER_CORE, D], F32, kind="ExternalInput")
    out = nc.dram_tensor("out", [SEG_PER_CORE, D], F32, kind="ExternalOutput")

    iota_c = nc.inline_tensor(
        np.tile(np.arange(W_WIN, dtype=np.float32), (128, 1)).astype(
            mybir.dt.np(BF16)), name="iota_c")
    ident_c = nc.inline_tensor(np.eye(128, dtype=np.float32), name="ident_c")
    ones_c = nc.inline_tensor(np.ones((128, 2), dtype=np.float32).astype(
        mybir.dt.np(BF16)), name="ones_c")
    zeros_c = nc.inline_tensor(np.zeros((129, D + 1), dtype=np.float32),
                               name="zeros_c")
    zeros_bf = nc.inline_tensor(
        np.zeros((129, D + 1), dtype=np.float32), name="zeros_bf")
    sel_c = nc.inline_tensor(
        np.tile(np.eye(SEG_PER_CORE, dtype=np.float32), (4, 1)).astype(
            mybir.dt.np(BF16)), name="sel_c")
    CCDT = BF16 if RS_BF16 else F32

    # [n_sup, 128, 8, 512]; per (t, p) the (8, 512) block is 16KB contiguous
    x_r = x.ap().rearrange("(t p k) d -> t p k d", p=128, k=8)
    RG = [list(range(N_CORES))]

    with tile.TileContext(nc) as tc:
        with tc.tile_pool(name="const", bufs=1) as const, \
             tc.tile_pool(name="dram", bufs=1, space="DRAM") as dram:
            iota_sb = const.tile([128, W_WIN], BF16, name="iota_sb")
            batch_sb = const.tile([128, 8 * n_sup], F32, name="batch_sb")
            idx_sb = const.tile([128, 1], I32, name="idx_sb")
            ones_sb = const.tile([128, 2], BF16, name="ones_sb")
            wt_sb = const.tile([128, 4 * D], BF16, name="wt_sb")
            b_sb = const.tile([SEG_PER_CORE, D], F32, name="b_sb")
            ohacc = const.tile([128, W_WIN], BF16, name="ohacc")
            nc.sync.dma_start(out=iota_sb[:, :], in_=iota_c[:, :])
            nc.sync.dma_start(out=batch_sb[:, :], in_=batchp[:, :])
            nc.gpsimd.dma_start(out=idx_sb[:, :], in_=idx[:, :])
            nc.gpsimd.dma_start(out=ones_sb[:, :], in_=ones_c[:, :])
            nc.gpsimd.dma_start(
                out=ohacc[:, :],
                in_=zeros_bf[0:128, 0:W_WIN // 2].bitcast(BF16))

            rs_in = dram.tile([N_SEG + 1, D + 1], CCDT, name="rs_in")
            rs_out = dram.tile([N_SEG, D + 1], CCDT, name="rs_out")
            zsrc = zeros_bf.ap().bitcast(CCDT) if RS_BF16 else zeros_c.ap()
            # zero the scatter target (only 128 of 257 rows get data)
            nc.gpsimd.dma_start(out=rs_in[0:129, :],
                                 in_=zsrc[0:129, 0:D + 1])
            nc.gpsimd.dma_start(out=rs_in[129:257, :],
                                in_=zsrc[0:128, 0:D + 1])

            # fire-and-forget tiny AllReduce: warms the ncfw collective
            # path (cold doorbell->poll costs ~12us) while the main loop
            # streams; nothing depends on its output
            warm_in = dram.tile([128, 2], F32, name="warm_in")
            warm_out = dram.tile([128, 2], F32, name="warm_out",
                                 addr_space="Shared")
            warm_dma = nc.gpsimd.dma_start(out=warm_in[:, :],
                                           in_=zeros_c[0:128, 0:2])
            warm_cc = nc.gpsimd.collective_compute(
                "AllReduce", mybir.AluOpType.add, replica_groups=RG,
                ins=[warm_in.opt()], outs=[warm_out.opt()])
            bass._add_dep_helper(warm_cc.ins, warm_dma.ins, False,
                                 "warm AR right after its input")

            with tc.tile_pool(name="xin", bufs=4) as xp, \
                 tc.tile_pool(name="ohp", bufs=12) as ohp, \
                 tc.tile_pool(name="psum_acc", bufs=1, space="PSUM") as pacc:
                ps = pacc.tile([128, D], F32, name="ps")
                for t in range(n_sup):
                    xt = xp.tile([128, 8, D], F32, name="xt")
                    xq = nc.sync if t % 2 == 0 else nc.scalar
                    if t == n_sup - 1:
                        # split the final supertile so the tail of the
                        # pipeline drains per-2-plane, not per-8-plane
                        for c in range(4):
                            q2 = nc.sync if c % 2 == 0 else nc.scalar
                            q2.dma_start(out=xt[:, 2 * c:2 * c + 2, :],
                                         in_=x_r[t][:, 2 * c:2 * c + 2, :])
                    else:
                        xq.dma_start(out=xt[:, :, :], in_=x_r[t])
                    # little-endian f32: the high halfword of each element
                    # IS its truncated bf16 value -> free bf16 operand
                    xt_bf = xt[:, :, :].bitcast(BF16)
                    st, sp0 = (t == 0), (t == n_sup - 1)
                    for k in range(8):
                        oh = ohp.tile([128, W_WIN], BF16, name="oh")
                        nc.vector.tensor_scalar(
                            oh[:, :], iota_sb[:, :],
                            batch_sb[:, k * n_sup + t:k * n_sup + t + 1],
                            None, mybir.AluOpType.is_equal)
                        nc.tensor.matmul(ps[:, :], oh[:, :],
                                         xt_bf[:, k, 1::2],
                                         start=(st and k == 0),
                                         stop=(sp0 and k == 7),
                                         skip_group_check=True)
                        # counts only need the first 64 window columns
                        # (per-core span is ~33; host asserts <= 64);
                        # same engine as is_eq so the accumulator never
                        # drains late
                        nc.vector.tensor_tensor(
                            ohacc[:, 0:64], ohacc[:, 0:64],
                            oh[:, 0:64], mybir.AluOpType.add)

                with tc.tile_pool(name="half", bufs=1) as halfp, \
                     tc.tile_pool(name="psum_cnt", bufs=1,
                                  space="PSUM") as pcnt:
                    # counts: column sums of the accumulated one-hots
                    pc = pcnt.tile([128, 2], F32, name="pc")
                    nc.tensor.matmul(pc[:, :], ohacc[:, :], ones_sb[:, :],
                                     start=True, stop=True)
                    sbw = halfp.tile([128, D + 1], CCDT, name="sbw")
                    nc.vector.tensor_copy(sbw[:, 0:D // 2], ps[:, 0:D // 2])
                    nc.scalar.copy(sbw[:, D // 2:D], ps[:, D // 2:D])
                    nc.scalar.copy(sbw[:, D:D + 1], pc[:, 0:1])
                    # place the local window at its global segment rows
                    # (row p -> rs_in[idx[p]]; idx clamps overflow to the
                    # dummy row 256)
                    nc.gpsimd.indirect_dma_start(
                        out=rs_in[:, :], out_offset=bass.IndirectOffsetOnAxis(
                            ap=idx_sb[:, 0:1], axis=0),
                        in_=sbw[:, :], in_offset=None)

            # AllToAll (pure copy, ~2x the wire rate of ReduceScatter):
            # block i of the output is rank i's partial for MY 32 segments;
            # the 8-way sum happens on-core right after
            nc.gpsimd.collective_compute(
                "AllToAll", mybir.AluOpType.bypass, replica_groups=RG,
                ins=[rs_in[0:N_SEG, :].opt()], outs=[rs_out.opt()])

            # epilogue inputs (only needed after the collective)
            for i in range(4):
                nc.scalar.dma_start(out=wt_sb[:, i * D:(i + 1) * D],
                                    in_=wt[i * 128:(i + 1) * 128, :])
            nc.scalar.dma_start(out=b_sb[:, :], in_=bb[:, :])

            with tc.tile_pool(name="epi", bufs=1) as epi, \
                 tc.tile_pool(name="psum_epi", bufs=1, space="PSUM") as pepi:
                sel_sb = epi.tile([128, SEG_PER_CORE], BF16, name="sel_sb")
                nc.scalar.dma_start(out=sel_sb[:, :], in_=sel_c[:, :])
                # my 32 segments' partials from all 8 ranks: rows (i*32+s)
                blk0 = epi.tile([128, D + 1], CCDT, name="blk0")
                blk1 = epi.tile([128, D + 1], CCDT, name="blk1")
                nc.sync.dma_start(out=blk0[:, :], in_=rs_out[0:128, :])
                nc.scalar.dma_start(out=blk1[:, :], in_=rs_out[128:256, :])

                # fused 8-block sum + transpose on the TensorEngine:
                # pt_c[d, s] = sum_p blk[p, d_c] * sel[p, s]
                lhsT = epi.tile([128, 4 * SEG_PER_CORE], BF16, name="lhsT")
                for c in range(4):
                    pt = pepi.tile([128, SEG_PER_CORE], F32, name="pt",
                                   tag="pt", bufs=2)
                    nc.tensor.matmul(pt[:, :],
                                     blk0[:, c * 128:(c + 1) * 128],
                                     sel_sb[:, :], start=True, stop=False)
                    nc.tensor.matmul(pt[:, :],
                                     blk1[:, c * 128:(c + 1) * 128],
                                     sel_sb[:, :], start=False, stop=True)
                    if c % 2 == 0:
                        nc.vector.tensor_copy(
                            lhsT[:, c * SEG_PER_CORE:(c + 1) * SEG_PER_CORE],
                            pt[:, :])
                    else:
                        nc.scalar.copy(
                            lhsT[:, c * SEG_PER_CORE:(c + 1) * SEG_PER_CORE],
                            pt[:, :])
                # counts: cnt[s] = sum_p sel[p, s] * blk[p, 512]
                pcnt2 = pepi.tile([SEG_PER_CORE, 1], F32, name="pcnt2")
                nc.tensor.matmul(pcnt2[:, :], sel_sb[:, :],
                                 blk0[:, D:D + 1], start=True, stop=False)
                nc.tensor.matmul(pcnt2[:, :], sel_sb[:, :],
                                 blk1[:, D:D + 1], start=False, stop=True)

                cm = epi.tile([SEG_PER_CORE, 1], F32, name="cm")
                inv = epi.tile([SEG_PER_CORE, 1], F32, name="inv")
                ind = epi.tile([SEG_PER_CORE, 1], F32, name="ind")
                bind = epi.tile([SEG_PER_CORE, D], F32, name="bind")
                nc.vector.tensor_scalar_max(cm[:, :], pcnt2[:, :], 1.0)
                nc.vector.reciprocal(inv[:, :], cm[:, :])
                nc.vector.tensor_scalar_min(ind[:, :], pcnt2[:, :], 1.0)
                nc.vector.tensor_scalar(bind[:, :], b_sb[:, :], ind[:, 0:1],
                                        None, mybir.AluOpType.mult)

                po = pepi.tile([SEG_PER_CORE, D], F32, name="po")
                for i in range(4):
                    nc.tensor.matmul(
                        po[:, :],
                        lhsT[:, i * SEG_PER_CORE:(i + 1) * SEG_PER_CORE],
                        wt_sb[:, i * D:(i + 1) * D],
                        start=(i == 0), stop=(i == 3))
                res = epi.tile([SEG_PER_CORE, D], F32, name="res")
                # res = (sums @ Wt) / max(c,1) + b*min(c,1)
                nc.vector.scalar_tensor_tensor(
                    res[:, :], po[:, :], inv[:, 0:1],
                    bind[:, :], mybir.AluOpType.mult, mybir.AluOpType.add)
                nc.sync.dma_start(out=out[:, :], in_=res[:, :])
    nc.compile()
    return nc


def make_in_maps(x, W, b, batch, rows_per_core):
    x = np.asarray(x, dtype=np.float32)
    W = np.asarray(W, dtype=np.float32)
    b = np.asarray(b, dtype=np.float32)
    batch = np.asarray(batch)
    n_sup = rows_per_core // 1024
    wt = np.ascontiguousarray(W.T).astype(mybir.dt.np(BF16))
    bb = np.ascontiguousarray(np.tile(b.reshape(1, D), (SEG_PER_CORE, 1)))
    in_maps = []
    for j in range(N_CORES):
        lo = j * rows_per_core
        bs = batch[lo:lo + rows_per_core].astype(np.int64)
        base = int(bs[0])
        rel = (bs - base).astype(np.float32)
        assert rel.max() < W_WIN, (
            f"core {j}: segment span {int(rel.max()) + 1} exceeds window")
        assert rel.max() < 64, f"core {j}: span {int(rel.max()) + 1} > 64"
        planes = rel.reshape(n_sup, 128, 8)
        bp = np.concatenate([planes[:, :, k].T for k in range(8)], axis=1)
        rowidx = np.minimum(base + np.arange(128), N_SEG).astype(np.int32)
        in_maps.append({
            "x": np.ascontiguousarray(x[lo:lo + rows_per_core]),
            "batchp": np.ascontiguousarray(bp),
            "idx": np.ascontiguousarray(rowidx.reshape(128, 1)),
            "wt": wt,
            "bb": bb,
        })
    return in_maps


_NC_CACHE = {}


def kernel(x, W, b, batch, num_segments, trace=False):
    assert int(num_segments) == N_SEG
    rows_per_core = N_ROWS // N_CORES
    if rows_per_core not in _NC_CACHE:
        _NC_CACHE[rows_per_core] = build_nc(rows_per_core)
    nc = _NC_CACHE[rows_per_core]
    in_maps = make_in_maps(x, W, b, batch, rows_per_core)
    res = run_bass_kernel_spmd(nc, in_maps, core_ids=list(range(N_CORES)),
                               trace=trace)
    full = np.concatenate([res.results[j]["out"] for j in range(N_CORES)],
                          axis=0)
    if trace:
        return full, res
    return full



# revision 2
# speedup vs baseline: 1.3339x; 1.3339x over previous
"""Distributed Trainium2 kernel for AlternateWeaveGather (segment_reduce).

Reference computation:
    h = x @ W.T + b                      # [N, 512] linear
    out = segment_mean(h, batch, 256)    # [256, 512]

Since the linear layer commutes with the segment sum:
    out[s] = (segsum_x[s] @ W.T) / max(c[s], 1) + b * (c[s] > 0)

each core segment-reduces its row shard of x (one-hot matmul on the
TensorEngine); batch is sorted, so only the first/last segment of a
core's contiguous ~33-segment window can straddle a core boundary.
Cross-core traffic is a [32, 520] f32 AllToAll (4 rows per peer:
head/tail boundary partials + counts; rows padded to 2080B for 32B
DMA alignment).

x ships as bf16 (the matmul consumes bf16 anyway; host round-to-nearest
beats on-device truncation), halving HBM volume. The row stream leads
with two tiny 256-row boundary tiles and ramps tile sizes up
(256,256,512,1024,2048...) so the first tiles land in ~1us instead of
waiting out a deep 2MB pipeline fill; the exchange collective fires
~15us in and completes in the shadow of the remaining stream. All
exchange staging copies run on the Vector queue so the Scalar queue
never stalls the x stream.

Sharding: data-parallel over rows. x/batch split along dim 0 across 8
cores; W/b replicated; host concatenates the 8x[32, 512] outputs.
"""

import numpy as np

import concourse.bacc as bacc
import concourse.bass as bass
import concourse.mybir as mybir
import concourse.tile as tile
from concourse.bass_utils import run_bass_kernel_spmd

N_CORES = 8
N_ROWS = 131072
D = 512
DP = D + 8   # exchange row width, padded to 32B alignment (2080B)
N_SEG = 256
SEG_PER_CORE = N_SEG // N_CORES
W_WIN = 64   # one-hot window width (per-core segment span is ~33)
EX = 4       # exchange rows per neighbor block (boundary segs per side)
BND = 256    # boundary tile rows (head/tail rows per core must fit)

# row-stream tiling: (start_row, n_rows) per tile, boundary tiles first
def _tiles(rows_per_core):
    ts = [(rows_per_core - BND, BND), (0, BND)]
    pos, size = BND, 2 * BND
    while pos < rows_per_core - BND:
        size = min(size, 2048, rows_per_core - BND - pos)
        ts.append((pos, size))
        pos += size
        size *= 2
    return ts

F32 = mybir.dt.float32
I32 = mybir.dt.int32
BF16 = mybir.dt.bfloat16


def build_nc(rows_per_core=N_ROWS // N_CORES):
    tiles = _tiles(rows_per_core)
    n_planes = sum(r for _, r in tiles) // 128
    assert sum(r for _, r in tiles) == rows_per_core

    nc = bacc.Bacc("TRN2", target_bir_lowering=False, debug=False,
                   num_devices=N_CORES)
    x = nc.dram_tensor("x", [rows_per_core, D], BF16, kind="ExternalInput")
    # batchp[p, c] = batch_rel[row(tile, plane k, partition p)], c in
    # flat processing-order plane index
    batchp = nc.dram_tensor("batchp", [128, n_planes], F32,
                            kind="ExternalInput")
    scat = nc.dram_tensor("scat", [W_WIN, 1], I32, kind="ExternalInput")
    selmain = nc.dram_tensor("selmain", [W_WIN, SEG_PER_CORE], BF16,
                             kind="ExternalInput")
    selfull = nc.dram_tensor("selfull", [8 * EX, SEG_PER_CORE], BF16,
                             kind="ExternalInput")
    wt = nc.dram_tensor("wt", [D, D], BF16, kind="ExternalInput")
    bb = nc.dram_tensor("bb", [SEG_PER_CORE, D], F32, kind="ExternalInput")
    out = nc.dram_tensor("out", [SEG_PER_CORE, D], F32, kind="ExternalOutput")

    iota_c = nc.inline_tensor(
        np.tile(np.arange(W_WIN, dtype=np.float32), (128, 1)).astype(
            mybir.dt.np(BF16)), name="iota_c")
    # counts matmul rhs: col 0 ones, cols 1-7 zero (pads sbw col 512:520)
    cnt8 = np.zeros((128, 8), dtype=np.float32)
    cnt8[:, 0] = 1.0
    cnt8_c = nc.inline_tensor(cnt8.astype(mybir.dt.np(BF16)), name="cnt8_c")
    zeros_c = nc.inline_tensor(np.zeros((129, DP + 8), dtype=np.float32),
                               name="zeros_c")
    RG = [list(range(N_CORES))]

    with tile.TileContext(nc) as tc:
        with tc.tile_pool(name="const", bufs=1) as const, \
             tc.tile_pool(name="dram", bufs=1, space="DRAM") as dram, \
             tc.tile_pool(name="psum_acc", bufs=1, space="PSUM") as pacc:
            iota_sb = const.tile([128, W_WIN], BF16, name="iota_sb")
            batch_sb = const.tile([128, n_planes], F32, name="batch_sb")
            scat_sb = const.tile([W_WIN, 1], I32, name="scat_sb")
            cnt8_sb = const.tile([128, 8], BF16, name="cnt8_sb")
            ohacc_b = const.tile([128, W_WIN], BF16, name="ohacc_b")
            ohacc_m = const.tile([128, W_WIN], BF16, name="ohacc_m")
            selmain_sb = const.tile([W_WIN, SEG_PER_CORE], BF16,
                                    name="selmain_sb")
            selfull_sb = const.tile([8 * EX, SEG_PER_CORE], BF16,
                                    name="selfull_sb")
            wt_sb = const.tile([128, 4 * D], BF16, name="wt_sb")
            b_sb = const.tile([SEG_PER_CORE, D], F32, name="b_sb")
            sbw_b = const.tile([W_WIN, DP], F32, name="sbw_b")
            sbw_bh = const.tile([W_WIN, DP], BF16, name="sbw_bh")
            sbw_m = const.tile([W_WIN, DP], BF16, name="sbw_m")
            recv_sb = const.tile([8 * EX, DP], BF16, name="recv_sb")
            lhsT = const.tile([128, 4 * SEG_PER_CORE], BF16, name="lhsT")

            # fire-and-forget warm AllReduce FIRST on gpsimd: its doorbell
            # must land before ncfw's first doorbell poll (~19us in).
            # Its mesh runs ~20-35us and leaves ncfw hot, so the real
            # exchange doorbell (~30us) is picked up immediately instead
            # of waiting ~70us for the next poll cycle.
            warm_in = dram.tile([8, 8], F32, name="warm_in")
            warm_out = dram.tile([8, 8], F32, name="warm_out",
                                 addr_space="Shared")
            warm_dma = nc.gpsimd.dma_start(out=warm_in[:, :],
                                           in_=zeros_c[0:8, 0:8])
            warm_cc = nc.gpsimd.collective_compute(
                "AllReduce", mybir.AluOpType.add, replica_groups=RG,
                ins=[warm_in.opt()], outs=[warm_out.opt()])
            bass._add_dep_helper(warm_cc.ins, warm_dma.ins, False,
                                 "warm AR right after its input")

            # stream-critical consts on the sync queue, ahead of x
            nc.sync.dma_start(out=iota_sb[:, :], in_=iota_c[:, :])
            nc.sync.dma_start(out=batch_sb[:, :], in_=batchp[:, :])
            # everything else on the (otherwise idle) gpsimd queue;
            # ohacc zero fills cover the full W_WIN width so the count
            # matmuls see zeros beyond the accumulated columns
            zbf = zeros_c.ap().bitcast(BF16)
            nc.gpsimd.dma_start(out=ohacc_b[:, :], in_=zbf[0:128, 0:W_WIN])
            nc.gpsimd.dma_start(out=ohacc_m[:, :],
                                in_=zbf[0:128, W_WIN:2 * W_WIN])
            nc.gpsimd.dma_start(out=scat_sb[:, :], in_=scat[:, :])
            nc.gpsimd.dma_start(out=cnt8_sb[:, :], in_=cnt8_c[:, :])

            # exchange buffers: last row of a2a_in is the trash row for
            # non-exchange window rows; rows 0:8*EX are per-peer blocks
            a2a_in = dram.tile([8 * EX + 1, DP], F32, name="a2a_in")
            a2a_out = dram.tile([8 * EX, DP], F32, name="a2a_out")
            nc.gpsimd.dma_start(out=a2a_in[:, :],
                                in_=zeros_c[0:8 * EX + 1, 0:DP])

            with tc.tile_pool(name="xin", bufs=4) as xp, \
                 tc.tile_pool(name="ohp", bufs=12) as ohp:
                ps_m = pacc.tile([W_WIN, D], F32, name="ps_m")
                ps_b = pacc.tile([W_WIN, D], F32, name="ps_b")
                cflat = 0
                for i, (row0, nrows) in enumerate(tiles):
                    kp = nrows // 128
                    xin = x.ap()[row0:row0 + nrows, :].rearrange(
                        "(p k) d -> p k d", p=128, k=kp)
                    xt = xp.tile([128, 16, D], BF16, name="xt")
                    xq = nc.sync if i % 2 == 0 else nc.scalar
                    if i == len(tiles) - 1:
                        # split the final tile so the pipeline tail
                        # drains per-quarter
                        for c in range(4):
                            q2 = nc.sync if c % 2 == 0 else nc.scalar
                            kq = kp // 4
                            q2.dma_start(
                                out=xt[:, c * kq:(c + 1) * kq, :],
                                in_=xin[:, c * kq:(c + 1) * kq, :])
                    else:
                        xq.dma_start(out=xt[:, 0:kp, :], in_=xin)
                    bnd = i < 2
                    ps = ps_b if bnd else ps_m
                    acc = ohacc_b if bnd else ohacc_m
                    st = (i == 0) if bnd else (i == 2)
                    sp = (i == 1) if bnd else (i == len(tiles) - 1)
                    for k in range(kp):
                        oh = ohp.tile([128, W_WIN], BF16, name="oh")
                        nc.vector.tensor_scalar(
                            oh[:, :], iota_sb[:, :],
                            batch_sb[:, cflat:cflat + 1],
                            None, mybir.AluOpType.is_equal)
                        cflat += 1
                        nc.tensor.matmul(ps[:, :], oh[:, :],
                                         xt[:, k, :],
                                         start=(st and k == 0),
                                         stop=(sp and k == kp - 1),
                                         skip_group_check=True)
                        # counts: accumulate one-hots
                        nc.vector.tensor_tensor(
                            acc[:, :], acc[:, :],
                            oh[:, :], mybir.AluOpType.add)
                    if i == 1:
                        # boundary window complete: assemble + ship the
                        # exchange rows, fire the AllToAll. All staging
                        # stays on Vector/Tensor so Scalar keeps
                        # streaming x.
                        with tc.tile_pool(name="psum_cb", bufs=1,
                                          space="PSUM") as pcb:
                            pc_b = pcb.tile([W_WIN, 8], F32, name="pc_b")
                            nc.tensor.matmul(pc_b[:, :], ohacc_b[:, :],
                                             cnt8_sb[:, :], start=True,
                                             stop=True)
                            nc.vector.tensor_copy(sbw_b[:, 0:D],
                                                  ps_b[:, 0:D])
                            nc.vector.tensor_copy(sbw_b[:, D:DP],
                                                  pc_b[:, 0:8])
                        nc.gpsimd.indirect_dma_start(
                            out=a2a_in[:, :],
                            out_offset=bass.IndirectOffsetOnAxis(
                                ap=scat_sb[:, 0:1], axis=0),
                            in_=sbw_b[:, :], in_offset=None)
                        nc.gpsimd.collective_compute(
                            "AllToAll", mybir.AluOpType.bypass,
                            replica_groups=RG,
                            ins=[a2a_in[0:8 * EX, :].opt()],
                            outs=[a2a_out.opt()])
                        # epilogue inputs, on gpsimd behind the trigger
                        for ci in range(4):
                            nc.gpsimd.dma_start(
                                out=wt_sb[:, ci * D:(ci + 1) * D],
                                in_=wt[ci * 128:(ci + 1) * 128, :])
                        nc.gpsimd.dma_start(out=b_sb[:, :], in_=bb[:, :])
                        nc.gpsimd.dma_start(out=selmain_sb[:, :],
                                            in_=selmain[:, :])
                        nc.gpsimd.dma_start(out=selfull_sb[:, :],
                                            in_=selfull[:, :])
                        # receive side: cast f32 -> bf16 during the DMA;
                        # waits on the collective completion semaphore
                        nc.gpsimd.dma_start(out=recv_sb[:, :],
                                            in_=a2a_out[:, :])

            with tc.tile_pool(name="epi", bufs=1) as epi, \
                 tc.tile_pool(name="psum_epi", bufs=1,
                              space="PSUM") as pepi:
                pc_m = pepi.tile([W_WIN, 8], F32, name="pc_m")
                nc.tensor.matmul(pc_m[:, :], ohacc_m[:, :], cnt8_sb[:, :],
                                 start=True, stop=True)
                nc.vector.tensor_copy(sbw_m[:, 0:D], ps_m[:, 0:D])
                nc.scalar.copy(sbw_m[:, D:DP], pc_m[:, 0:8])
                nc.vector.tensor_copy(sbw_bh[:, :], sbw_b[:, :])

                # fused 3-source sum + transpose on the TensorEngine:
                # pt[d, s] = sum_p src[p, d] * sel[p, s]
                for c in range(4):
                    pt = pepi.tile([128, SEG_PER_CORE], F32, name="pt",
                                   tag="pt", bufs=2)
                    nc.tensor.matmul(pt[:, :],
                                     sbw_m[:, c * 128:(c + 1) * 128],
                                     selmain_sb[:, :], start=True,
                                     stop=False)
                    nc.tensor.matmul(pt[:, :],
                                     sbw_bh[:, c * 128:(c + 1) * 128],
                                     selmain_sb[:, :], start=False,
                                     stop=False)
                    nc.tensor.matmul(pt[:, :],
                                     recv_sb[:, c * 128:(c + 1) * 128],
                                     selfull_sb[:, :], start=False,
                                     stop=True)
                    if c % 2 == 0:
                        nc.vector.tensor_copy(
                            lhsT[:, c * SEG_PER_CORE:(c + 1) * SEG_PER_CORE],
                            pt[:, :])
                    else:
                        nc.scalar.copy(
                            lhsT[:, c * SEG_PER_CORE:(c + 1) * SEG_PER_CORE],
                            pt[:, :])
                # counts: cnt[s] = sum_p sel[p, s] * col512 of each source
                pcnt = pepi.tile([SEG_PER_CORE, 1], F32, name="pcnt")
                nc.tensor.matmul(pcnt[:, :], selmain_sb[:, :],
                                 sbw_m[:, D:D + 1], start=True, stop=False)
                nc.tensor.matmul(pcnt[:, :], selmain_sb[:, :],
                                 sbw_bh[:, D:D + 1], start=False, stop=False)
                nc.tensor.matmul(pcnt[:, :], selfull_sb[:, :],
                                 recv_sb[:, D:D + 1], start=False, stop=True)

                cm = epi.tile([SEG_PER_CORE, 1], F32, name="cm")
                inv = epi.tile([SEG_PER_CORE, 1], F32, name="inv")
                ind = epi.tile([SEG_PER_CORE, 1], F32, name="ind")
                bind = epi.tile([SEG_PER_CORE, D], F32, name="bind")
                nc.vector.tensor_scalar_max(cm[:, :], pcnt[:, :], 1.0)
                nc.vector.reciprocal(inv[:, :], cm[:, :])
                nc.vector.tensor_scalar_min(ind[:, :], pcnt[:, :], 1.0)
                nc.vector.tensor_scalar(bind[:, :], b_sb[:, :], ind[:, 0:1],
                                        None, mybir.AluOpType.mult)

                po = pepi.tile([SEG_PER_CORE, D], F32, name="po")
                for ci in range(4):
                    nc.tensor.matmul(
                        po[:, :],
                        lhsT[:, ci * SEG_PER_CORE:(ci + 1) * SEG_PER_CORE],
                        wt_sb[:, ci * D:(ci + 1) * D],
                        start=(ci == 0), stop=(ci == 3))
                res = epi.tile([SEG_PER_CORE, D], F32, name="res")
                # res = (sums @ Wt) / max(c,1) + b*min(c,1)
                nc.vector.scalar_tensor_tensor(
                    res[:, :], po[:, :], inv[:, 0:1],
                    bind[:, :], mybir.AluOpType.mult, mybir.AluOpType.add)
                nc.sync.dma_start(out=out[:, :], in_=res[:, :])
    nc.compile()
    return nc


def make_in_maps(x, W, b, batch, rows_per_core):
    x = np.asarray(x, dtype=np.float32)
    W = np.asarray(W, dtype=np.float32)
    b = np.asarray(b, dtype=np.float32)
    batch = np.asarray(batch).astype(np.int64)
    tiles = _tiles(rows_per_core)
    bf = mybir.dt.np(BF16)
    xh = np.ascontiguousarray(x.astype(bf))
    wt = np.ascontiguousarray(W.T).astype(bf)
    bb = np.ascontiguousarray(np.tile(b.reshape(1, D), (SEG_PER_CORE, 1)))

    bases, lasts = [], []
    for j in range(N_CORES):
        bs = batch[j * rows_per_core:(j + 1) * rows_per_core]
        bases.append(int(bs[0]))
        lasts.append(int(bs[-1]))

    in_maps = []
    for j in range(N_CORES):
        lo = j * rows_per_core
        bs = batch[lo:lo + rows_per_core]
        base, last = bases[j], lasts[j]
        span = last - base + 1
        own_lo, own_hi = SEG_PER_CORE * j, SEG_PER_CORE * (j + 1)
        head_segs = max(0, own_lo - base)
        tail_segs = max(0, last - own_hi + 1)
        head_rows = int(np.sum(bs < own_lo))
        tail_rows = int(np.sum(bs >= own_hi))
        assert span <= W_WIN, f"core {j}: segment span {span} > {W_WIN}"
        assert head_segs <= EX and tail_segs <= EX, (
            f"core {j}: boundary spill {head_segs}/{tail_segs} > {EX}")
        assert head_rows <= BND and tail_rows <= BND, (
            f"core {j}: boundary rows {head_rows}/{tail_rows} > {BND}")

        rel = (bs - base).astype(np.float32)
        cols = []
        for row0, nrows in tiles:
            blk = rel[row0:row0 + nrows].reshape(128, nrows // 128)
            cols.append(blk)
        bp = np.concatenate(cols, axis=1)

        # scatter map: window row p (seg g=base+p) -> a2a_in row
        sc = np.full(W_WIN, 8 * EX, dtype=np.int32)  # trash row
        for p in range(span):
            g = base + p
            if g < own_lo:
                sc[p] = EX * (j - 1) + p
            elif g >= own_hi:
                sc[p] = EX * (j + 1) + (g - own_hi)

        # selmain: window row p -> owned column
        selm = np.zeros((W_WIN, SEG_PER_CORE), dtype=np.float32)
        for p in range(span):
            col = base + p - own_lo
            if 0 <= col < SEG_PER_CORE:
                selm[p, col] = 1.0

        # selfull: received a2a row (EX*k + s) -> owned column
        self_ = np.zeros((8 * EX, SEG_PER_CORE), dtype=np.float32)
        if j > 0:  # tail spill from core j-1 lands at start of my block
            for s in range(max(0, lasts[j - 1] - own_lo + 1)):
                self_[EX * (j - 1) + s, s] = 1.0
        if j < N_CORES - 1:  # head spill from core j+1: end of my block
            for s in range(max(0, own_hi - bases[j + 1])):
                col = bases[j + 1] + s - own_lo
                assert 0 <= col < SEG_PER_CORE
                self_[EX * (j + 1) + s, col] = 1.0

        in_maps.append({
            "x": xh[lo:lo + rows_per_core],
            "batchp": np.ascontiguousarray(bp),
            "scat": np.ascontiguousarray(sc.reshape(W_WIN, 1)),
            "selmain": selm.astype(bf),
            "selfull": self_.astype(bf),
            "wt": wt,
            "bb": bb,
        })
    return in_maps


_NC_CACHE = {}


def kernel(x, W, b, batch, num_segments, trace=False):
    assert int(num_segments) == N_SEG
    rows_per_core = N_ROWS // N_CORES
    if rows_per_core not in _NC_CACHE:
        _NC_CACHE[rows_per_core] = build_nc(rows_per_core)
    nc = _NC_CACHE[rows_per_core]
    in_maps = make_in_maps(x, W, b, batch, rows_per_core)
    res = run_bass_kernel_spmd(nc, in_maps, core_ids=list(range(N_CORES)),
                               trace=trace)
    full = np.concatenate([res.results[j]["out"] for j in range(N_CORES)],
                          axis=0)
    if trace:
        return full, res
    return full


# revision 3
# speedup vs baseline: 1.3730x; 1.0293x over previous
"""Distributed Trainium2 kernel for AlternateWeaveGather (segment_reduce).

Reference computation:
    h = x @ W.T + b                      # [N, 512] linear
    out = segment_mean(h, batch, 256)    # [256, 512]

Since the linear layer commutes with the segment sum:
    out[s] = (segsum_x[s] @ W.T) / max(c[s], 1) + b * (c[s] > 0)

each core segment-reduces its row shard of x with a one-hot matmul on
the TensorEngine, then applies the tiny linear to its 32 owned
segments.

Sharding: batch is sorted, so rows are sharded at SEGMENT boundaries -
core j gets exactly the rows of segments [32j, 32j+32), padded with
no-match rows to a fixed shape. Every segment's rows live on exactly
one core, so there is NO cross-core communication at all (the ncfw
collective path costs ~80-100us of bootstrap latency regardless of
payload, so removing it beats any overlap scheme). x ships as bf16
(the matmul consumes bf16 anyway; host round-to-nearest beats
on-device truncation), halving HBM volume; the row stream ramps tile
sizes (256..2048) so compute starts early and drains with the DMA.
W/b replicated; host concatenates the 8x[32, 512] outputs.
"""

import numpy as np

import concourse.bacc as bacc
import concourse.bass as bass
import concourse.mybir as mybir
import concourse.tile as tile
from concourse.bass_utils import run_bass_kernel_spmd

N_CORES = 8
N_ROWS = 131072
D = 512
DP = D + 8
N_SEG = 256
SEG_PER_CORE = N_SEG // N_CORES
W_WIN = 32   # one-hot window = exactly the owned segments


def _tiles(pad_rows):
    ts, pos, size = [], 0, 256
    while pos < pad_rows:
        size = min(size, 2048, pad_rows - pos)
        ts.append((pos, size))
        pos += size
        size *= 2
    # small final tile so the pipeline tail drains quickly
    if ts[-1][1] > 512:
        s, n = ts.pop()
        ts.append((s, n - 512))
        ts.append((s + n - 512, 512))
    return ts


F32 = mybir.dt.float32
BF16 = mybir.dt.bfloat16


def build_nc(pad_rows):
    tiles = _tiles(pad_rows)
    n_planes = pad_rows // 128
    assert sum(r for _, r in tiles) == pad_rows

    nc = bacc.Bacc("TRN2", target_bir_lowering=False, debug=False,
                   num_devices=N_CORES)
    x = nc.dram_tensor("x", [pad_rows, D], BF16, kind="ExternalInput")
    # batchp[p, c] = batch_rel[row(tile, plane k, partition p)], c in
    # flat processing-order plane index; padding rows get 99 (no match)
    batchp = nc.dram_tensor("batchp", [128, n_planes], F32,
                            kind="ExternalInput")
    wt = nc.dram_tensor("wt", [D, D], BF16, kind="ExternalInput")
    bb = nc.dram_tensor("bb", [SEG_PER_CORE, D], F32, kind="ExternalInput")
    out = nc.dram_tensor("out", [SEG_PER_CORE, D], F32, kind="ExternalOutput")

    iota_c = nc.inline_tensor(
        np.tile(np.arange(W_WIN, dtype=np.float32), (128, 1)).astype(
            mybir.dt.np(BF16)), name="iota_c")
    # counts matmul rhs: col 0 ones, cols 1-7 zero (pads sbw col 512:520)
    cnt8 = np.zeros((128, 8), dtype=np.float32)
    cnt8[:, 0] = 1.0
    cnt8_c = nc.inline_tensor(cnt8.astype(mybir.dt.np(BF16)), name="cnt8_c")
    sel32_c = nc.inline_tensor(
        np.eye(SEG_PER_CORE, dtype=np.float32).astype(mybir.dt.np(BF16)),
        name="sel32_c")
    zeros_c = nc.inline_tensor(np.zeros((129, 64), dtype=np.float32),
                               name="zeros_c")

    with tile.TileContext(nc) as tc:
        with tc.tile_pool(name="const", bufs=1) as const, \
             tc.tile_pool(name="psum_acc", bufs=1, space="PSUM") as pacc:
            iota_sb = const.tile([128, W_WIN], BF16, name="iota_sb")
            batch_sb = const.tile([128, n_planes], F32, name="batch_sb")
            cnt8_sb = const.tile([128, 8], BF16, name="cnt8_sb")
            sel32_sb = const.tile([SEG_PER_CORE, SEG_PER_CORE], BF16,
                                  name="sel32_sb")
            ohacc = const.tile([128, W_WIN], BF16, name="ohacc")
            wt_sb = const.tile([128, 4 * D], BF16, name="wt_sb")
            b_sb = const.tile([SEG_PER_CORE, D], F32, name="b_sb")
            sbw = const.tile([SEG_PER_CORE, DP], BF16, name="sbw")
            lhsT = const.tile([128, 4 * SEG_PER_CORE], BF16, name="lhsT")

            # stream-critical consts on the sync queue, ahead of x
            nc.sync.dma_start(out=iota_sb[:, :], in_=iota_c[:, :])
            nc.sync.dma_start(out=batch_sb[:, :], in_=batchp[:, :])
            # everything else on the (otherwise idle) gpsimd queue
            zbf = zeros_c.ap().bitcast(BF16)
            nc.gpsimd.dma_start(out=ohacc[:, :], in_=zbf[0:128, 0:W_WIN])
            nc.gpsimd.dma_start(out=cnt8_sb[:, :], in_=cnt8_c[:, :])
            nc.gpsimd.dma_start(out=sel32_sb[:, :], in_=sel32_c[:, :])
            for ci in range(4):
                nc.gpsimd.dma_start(out=wt_sb[:, ci * D:(ci + 1) * D],
                                    in_=wt[ci * 128:(ci + 1) * 128, :])
            nc.gpsimd.dma_start(out=b_sb[:, :], in_=bb[:, :])

            with tc.tile_pool(name="xin", bufs=4) as xp, \
                 tc.tile_pool(name="ohp", bufs=12) as ohp:
                ps = pacc.tile([W_WIN, D], F32, name="ps")
                cflat = 0
                for i, (row0, nrows) in enumerate(tiles):
                    kp = nrows // 128
                    xin = x.ap()[row0:row0 + nrows, :].rearrange(
                        "(p k) d -> p k d", p=128, k=kp)
                    xt = xp.tile([128, 16, D], BF16, name="xt")
                    xq = nc.sync if i % 2 == 0 else nc.scalar
                    if i == len(tiles) - 1 and kp >= 4:
                        # split the final tile so the pipeline tail
                        # drains per-quarter
                        for c in range(4):
                            q2 = nc.sync if c % 2 == 0 else nc.scalar
                            kq = kp // 4
                            q2.dma_start(
                                out=xt[:, c * kq:(c + 1) * kq, :],
                                in_=xin[:, c * kq:(c + 1) * kq, :])
                    else:
                        xq.dma_start(out=xt[:, 0:kp, :], in_=xin)
                    for k in range(kp):
                        oh = ohp.tile([128, W_WIN], BF16, name="oh")
                        nc.vector.tensor_scalar(
                            oh[:, :], iota_sb[:, :],
                            batch_sb[:, cflat:cflat + 1],
                            None, mybir.AluOpType.is_equal)
                        cflat += 1
                        nc.tensor.matmul(ps[:, :], oh[:, :],
                                         xt[:, k, :],
                                         start=(i == 0 and k == 0),
                                         stop=(i == len(tiles) - 1
                                               and k == kp - 1),
                                         skip_group_check=True)
                        # counts: accumulate one-hots
                        nc.vector.tensor_tensor(
                            ohacc[:, :], ohacc[:, :],
                            oh[:, :], mybir.AluOpType.add)

            with tc.tile_pool(name="epi", bufs=1) as epi, \
                 tc.tile_pool(name="psum_epi", bufs=1,
                              space="PSUM") as pepi:
                pc = pepi.tile([W_WIN, 8], F32, name="pc")
                nc.tensor.matmul(pc[:, :], ohacc[:, :], cnt8_sb[:, :],
                                 start=True, stop=True)
                nc.vector.tensor_copy(sbw[:, 0:D], ps[:, 0:D])
                nc.scalar.copy(sbw[:, D:DP], pc[:, 0:8])

                # transpose on the TensorEngine: pt[d, s] = sbw[s, d]
                for c in range(4):
                    pt = pepi.tile([128, SEG_PER_CORE], F32, name="pt",
                                   tag="pt", bufs=2)
                    nc.tensor.matmul(pt[:, :],
                                     sbw[:, c * 128:(c + 1) * 128],
                                     sel32_sb[:, :], start=True, stop=True)
                    if c % 2 == 0:
                        nc.vector.tensor_copy(
                            lhsT[:, c * SEG_PER_CORE:(c + 1) * SEG_PER_CORE],
                            pt[:, :])
                    else:
                        nc.scalar.copy(
                            lhsT[:, c * SEG_PER_CORE:(c + 1) * SEG_PER_CORE],
                            pt[:, :])
                pcnt = pepi.tile([SEG_PER_CORE, 1], F32, name="pcnt")
                nc.tensor.matmul(pcnt[:, :], sel32_sb[:, :],
                                 sbw[:, D:D + 1], start=True, stop=True)

                cm = epi.tile([SEG_PER_CORE, 1], F32, name="cm")
                inv = epi.tile([SEG_PER_CORE, 1], F32, name="inv")
                ind = epi.tile([SEG_PER_CORE, 1], F32, name="ind")
                bind = epi.tile([SEG_PER_CORE, D], F32, name="bind")
                nc.vector.tensor_scalar_max(cm[:, :], pcnt[:, :], 1.0)
                nc.vector.reciprocal(inv[:, :], cm[:, :])
                nc.vector.tensor_scalar_min(ind[:, :], pcnt[:, :], 1.0)
                nc.vector.tensor_scalar(bind[:, :], b_sb[:, :], ind[:, 0:1],
                                        None, mybir.AluOpType.mult)

                po = pepi.tile([SEG_PER_CORE, D], F32, name="po")
                for ci in range(4):
                    nc.tensor.matmul(
                        po[:, :],
                        lhsT[:, ci * SEG_PER_CORE:(ci + 1) * SEG_PER_CORE],
                        wt_sb[:, ci * D:(ci + 1) * D],
                        start=(ci == 0), stop=(ci == 3))
                res = epi.tile([SEG_PER_CORE, D], F32, name="res")
                # res = (sums @ Wt) / max(c,1) + b*min(c,1)
                nc.vector.scalar_tensor_tensor(
                    res[:, :], po[:, :], inv[:, 0:1],
                    bind[:, :], mybir.AluOpType.mult, mybir.AluOpType.add)
                nc.sync.dma_start(out=out[:, :], in_=res[:, :])
    nc.compile()
    return nc


def make_in_maps(x, W, b, batch, pad_rows, bnd):
    x = np.asarray(x, dtype=np.float32)
    W = np.asarray(W, dtype=np.float32)
    b = np.asarray(b, dtype=np.float32)
    batch = np.asarray(batch).astype(np.int64)
    tiles = _tiles(pad_rows)
    bf = mybir.dt.np(BF16)
    xh = np.ascontiguousarray(x.astype(bf))
    wt = np.ascontiguousarray(W.T).astype(bf)
    bb = np.ascontiguousarray(np.tile(b.reshape(1, D), (SEG_PER_CORE, 1)))

    in_maps = []
    for j in range(N_CORES):
        lo, hi = int(bnd[j]), int(bnd[j + 1])
        n = hi - lo
        assert n <= pad_rows
        xj = np.zeros((pad_rows, D), dtype=bf)
        xj[0:n] = xh[lo:hi]
        rel = np.full(pad_rows, 99.0, dtype=np.float32)
        rel[0:n] = (batch[lo:hi] - SEG_PER_CORE * j).astype(np.float32)
        assert n == 0 or (rel[0:n].min() >= 0 and rel[0:n].max() < W_WIN)

        cols = []
        for row0, nrows in tiles:
            cols.append(rel[row0:row0 + nrows].reshape(128, nrows // 128))
        bp = np.concatenate(cols, axis=1)

        in_maps.append({
            "x": xj,
            "batchp": np.ascontiguousarray(bp),
            "wt": wt,
            "bb": bb,
        })
    return in_maps


_NC_CACHE = {}


def kernel(x, W, b, batch, num_segments, trace=False):
    assert int(num_segments) == N_SEG
    batch_np = np.asarray(batch).astype(np.int64)
    # shard at segment boundaries: core j owns segments [32j, 32j+32)
    bnd = np.searchsorted(batch_np, np.arange(0, N_SEG + 1, SEG_PER_CORE))
    pad_rows = int(-(-int(np.diff(bnd).max()) // 256) * 256)
    if pad_rows not in _NC_CACHE:
        _NC_CACHE[pad_rows] = build_nc(pad_rows)
    nc = _NC_CACHE[pad_rows]
    in_maps = make_in_maps(x, W, b, batch, pad_rows, bnd)
    res = run_bass_kernel_spmd(nc, in_maps, core_ids=list(range(N_CORES)),
                               trace=trace)
    full = np.concatenate([res.results[j]["out"] for j in range(N_CORES)],
                          axis=0)
    if trace:
        return full, res
    return full


# revision 4
# speedup vs baseline: 1.3986x; 1.0186x over previous
"""Distributed Trainium2 kernel for AlternateWeaveGather (segment_reduce).

Reference computation:
    h = x @ W.T + b                      # [N, 512] linear
    out = segment_mean(h, batch, 256)    # [256, 512]

Since the linear layer commutes with the segment sum:
    out[s] = (segsum_x[s] @ W.T) / max(c[s], 1) + b * (c[s] > 0)

each core segment-reduces its row shard of x with a one-hot matmul on
the TensorEngine, then applies the tiny linear to its 32 owned
segments.

Sharding: batch is sorted, so rows are sharded at SEGMENT boundaries -
core j gets exactly the rows of segments [32j, 32j+32), padded with
no-match rows to a fixed shape. Every segment's rows live on exactly
one core, so there is NO cross-core communication at all (the ncfw
collective path costs ~80-100us of bootstrap latency regardless of
payload, so removing it beats any overlap scheme). x ships as bf16
(the matmul consumes bf16 anyway; host round-to-nearest beats
on-device truncation), halving HBM volume; the row stream ramps tile
sizes (256..2048) so compute starts early and drains with the DMA.
W/b replicated; host concatenates the 8x[32, 512] outputs.
"""

import numpy as np

import concourse.bacc as bacc
import concourse.bass as bass
import concourse.mybir as mybir
import concourse.tile as tile
from concourse.bass_utils import run_bass_kernel_spmd

N_CORES = 8
N_ROWS = 131072
D = 512
DP = D + 8
N_SEG = 256
SEG_PER_CORE = N_SEG // N_CORES
W_WIN = 32   # one-hot window = exactly the owned segments


def _tiles(pad_rows):
    ts, pos, size = [], 0, 256
    while pos < pad_rows:
        size = min(size, 2048, pad_rows - pos)
        ts.append((pos, size))
        pos += size
        size *= 2
    # small final tile so the pipeline tail drains quickly
    if ts[-1][1] > 512:
        s, n = ts.pop()
        ts.append((s, n - 512))
        ts.append((s + n - 512, 512))
    return ts


F32 = mybir.dt.float32
BF16 = mybir.dt.bfloat16


def build_nc(pad_rows):
    tiles = _tiles(pad_rows)
    n_planes = pad_rows // 128
    assert sum(r for _, r in tiles) == pad_rows

    nc = bacc.Bacc("TRN2", target_bir_lowering=False, debug=False,
                   num_devices=N_CORES)
    x = nc.dram_tensor("x", [pad_rows, D], BF16, kind="ExternalInput")
    # batchp[p, c] = batch_rel[row(tile, plane k, partition p)], c in
    # flat processing-order plane index; padding rows get 99 (no match)
    batchp = nc.dram_tensor("batchp", [128, n_planes], F32,
                            kind="ExternalInput")
    wt = nc.dram_tensor("wt", [D, D], BF16, kind="ExternalInput")
    bb = nc.dram_tensor("bb", [SEG_PER_CORE, D], F32, kind="ExternalInput")
    out = nc.dram_tensor("out", [SEG_PER_CORE, D], F32, kind="ExternalOutput")

    iota_c = nc.inline_tensor(
        np.tile(np.arange(W_WIN, dtype=np.float32), (128, 1)).astype(
            mybir.dt.np(BF16)), name="iota_c")
    # counts matmul rhs: col 0 ones, cols 1-7 zero (pads sbw col 512:520)
    cnt8 = np.zeros((128, 8), dtype=np.float32)
    cnt8[:, 0] = 1.0
    cnt8_c = nc.inline_tensor(cnt8.astype(mybir.dt.np(BF16)), name="cnt8_c")
    sel32_c = nc.inline_tensor(
        np.eye(SEG_PER_CORE, dtype=np.float32).astype(mybir.dt.np(BF16)),
        name="sel32_c")
    zeros_c = nc.inline_tensor(np.zeros((129, 64), dtype=np.float32),
                               name="zeros_c")

    with tile.TileContext(nc) as tc:
        with tc.tile_pool(name="const", bufs=1) as const, \
             tc.tile_pool(name="psum_acc", bufs=1, space="PSUM") as pacc:
            iota_sb = const.tile([128, W_WIN], BF16, name="iota_sb")
            batch_sb = const.tile([128, n_planes], F32, name="batch_sb")
            cnt8_sb = const.tile([128, 8], BF16, name="cnt8_sb")
            sel32_sb = const.tile([SEG_PER_CORE, SEG_PER_CORE], BF16,
                                  name="sel32_sb")
            ohacc = const.tile([128, W_WIN], BF16, name="ohacc")
            wt_sb = const.tile([128, 4 * D], BF16, name="wt_sb")
            b_sb = const.tile([SEG_PER_CORE, D], F32, name="b_sb")
            sbw = const.tile([SEG_PER_CORE, DP], BF16, name="sbw")
            lhsT = const.tile([128, 4 * SEG_PER_CORE], BF16, name="lhsT")

            # stream-critical consts on the sync queue, ahead of x
            nc.sync.dma_start(out=iota_sb[:, :], in_=iota_c[:, :])
            nc.sync.dma_start(out=batch_sb[:, :], in_=batchp[:, :])
            # everything else on the (otherwise idle) gpsimd queue
            zbf = zeros_c.ap().bitcast(BF16)
            nc.gpsimd.dma_start(out=ohacc[:, :], in_=zbf[0:128, 0:W_WIN])
            nc.gpsimd.dma_start(out=cnt8_sb[:, :], in_=cnt8_c[:, :])
            nc.gpsimd.dma_start(out=sel32_sb[:, :], in_=sel32_c[:, :])
            for ci in range(4):
                nc.gpsimd.dma_start(out=wt_sb[:, ci * D:(ci + 1) * D],
                                    in_=wt[ci * 128:(ci + 1) * 128, :])
            nc.gpsimd.dma_start(out=b_sb[:, :], in_=bb[:, :])

            with tc.tile_pool(name="xin", bufs=4) as xp, \
                 tc.tile_pool(name="ohp", bufs=12) as ohp:
                ps = pacc.tile([W_WIN, D], F32, name="ps")
                cflat = 0
                for i, (row0, nrows) in enumerate(tiles):
                    kp = nrows // 128
                    xin = x.ap()[row0:row0 + nrows, :].rearrange(
                        "(p k) d -> p k d", p=128, k=kp)
                    xt = xp.tile([128, 16, D], BF16, name="xt")
                    xq = nc.sync if i % 2 == 0 else nc.scalar
                    if i >= len(tiles) - 3 and kp >= 4:
                        # split the trailing tiles' DMAs so planes become
                        # consumable incrementally - a monolithic 2MB DMA
                        # completes all-or-nothing and leaves the tensor
                        # engine a ~16-plane backlog at stream end
                        for c in range(4):
                            q2 = nc.sync if c % 2 == 0 else nc.scalar
                            kq = kp // 4
                            q2.dma_start(
                                out=xt[:, c * kq:(c + 1) * kq, :],
                                in_=xin[:, c * kq:(c + 1) * kq, :])
                    else:
                        xq.dma_start(out=xt[:, 0:kp, :], in_=xin)
                    for k in range(kp):
                        oh = ohp.tile([128, W_WIN], BF16, name="oh")
                        nc.vector.tensor_scalar(
                            oh[:, :], iota_sb[:, :],
                            batch_sb[:, cflat:cflat + 1],
                            None, mybir.AluOpType.is_equal)
                        cflat += 1
                        nc.tensor.matmul(ps[:, :], oh[:, :],
                                         xt[:, k, :],
                                         start=(i == 0 and k == 0),
                                         stop=(i == len(tiles) - 1
                                               and k == kp - 1),
                                         skip_group_check=True)
                        # counts: accumulate one-hots
                        nc.vector.tensor_tensor(
                            ohacc[:, :], ohacc[:, :],
                            oh[:, :], mybir.AluOpType.add)

            with tc.tile_pool(name="epi", bufs=1) as epi, \
                 tc.tile_pool(name="psum_epi", bufs=1,
                              space="PSUM") as pepi:
                pc = pepi.tile([W_WIN, 8], F32, name="pc")
                nc.tensor.matmul(pc[:, :], ohacc[:, :], cnt8_sb[:, :],
                                 start=True, stop=True)
                nc.vector.tensor_copy(sbw[:, 0:D], ps[:, 0:D])
                nc.scalar.copy(sbw[:, D:DP], pc[:, 0:8])

                # transpose on the TensorEngine: pt[d, s] = sbw[s, d]
                for c in range(4):
                    pt = pepi.tile([128, SEG_PER_CORE], F32, name="pt",
                                   tag="pt", bufs=2)
                    nc.tensor.matmul(pt[:, :],
                                     sbw[:, c * 128:(c + 1) * 128],
                                     sel32_sb[:, :], start=True, stop=True)
                    if c % 2 == 0:
                        nc.vector.tensor_copy(
                            lhsT[:, c * SEG_PER_CORE:(c + 1) * SEG_PER_CORE],
                            pt[:, :])
                    else:
                        nc.scalar.copy(
                            lhsT[:, c * SEG_PER_CORE:(c + 1) * SEG_PER_CORE],
                            pt[:, :])
                pcnt = pepi.tile([SEG_PER_CORE, 1], F32, name="pcnt")
                nc.tensor.matmul(pcnt[:, :], sel32_sb[:, :],
                                 sbw[:, D:D + 1], start=True, stop=True)

                cm = epi.tile([SEG_PER_CORE, 1], F32, name="cm")
                inv = epi.tile([SEG_PER_CORE, 1], F32, name="inv")
                ind = epi.tile([SEG_PER_CORE, 1], F32, name="ind")
                bind = epi.tile([SEG_PER_CORE, D], F32, name="bind")
                nc.vector.tensor_scalar_max(cm[:, :], pcnt[:, :], 1.0)
                nc.vector.reciprocal(inv[:, :], cm[:, :])
                nc.vector.tensor_scalar_min(ind[:, :], pcnt[:, :], 1.0)
                nc.vector.tensor_scalar(bind[:, :], b_sb[:, :], ind[:, 0:1],
                                        None, mybir.AluOpType.mult)

                po = pepi.tile([SEG_PER_CORE, D], F32, name="po")
                for ci in range(4):
                    nc.tensor.matmul(
                        po[:, :],
                        lhsT[:, ci * SEG_PER_CORE:(ci + 1) * SEG_PER_CORE],
                        wt_sb[:, ci * D:(ci + 1) * D],
                        start=(ci == 0), stop=(ci == 3))
                res = epi.tile([SEG_PER_CORE, D], F32, name="res")
                # res = (sums @ Wt) / max(c,1) + b*min(c,1); split halves
                # across engines/queues so the closing chain overlaps
                nc.vector.scalar_tensor_tensor(
                    res[:, 0:D // 2], po[:, 0:D // 2], inv[:, 0:1],
                    bind[:, 0:D // 2], mybir.AluOpType.mult,
                    mybir.AluOpType.add)
                nc.sync.dma_start(out=out[:, 0:D // 2],
                                  in_=res[:, 0:D // 2])
                nc.vector.scalar_tensor_tensor(
                    res[:, D // 2:D], po[:, D // 2:D], inv[:, 0:1],
                    bind[:, D // 2:D], mybir.AluOpType.mult,
                    mybir.AluOpType.add)
                nc.scalar.dma_start(out=out[:, D // 2:D],
                                    in_=res[:, D // 2:D])
    nc.compile()
    return nc


def make_in_maps(x, W, b, batch, pad_rows, bnd):
    x = np.asarray(x, dtype=np.float32)
    W = np.asarray(W, dtype=np.float32)
    b = np.asarray(b, dtype=np.float32)
    batch = np.asarray(batch).astype(np.int64)
    tiles = _tiles(pad_rows)
    bf = mybir.dt.np(BF16)
    xh = np.ascontiguousarray(x.astype(bf))
    wt = np.ascontiguousarray(W.T).astype(bf)
    bb = np.ascontiguousarray(np.tile(b.reshape(1, D), (SEG_PER_CORE, 1)))

    in_maps = []
    for j in range(N_CORES):
        lo, hi = int(bnd[j]), int(bnd[j + 1])
        n = hi - lo
        assert n <= pad_rows
        xj = np.zeros((pad_rows, D), dtype=bf)
        xj[0:n] = xh[lo:hi]
        rel = np.full(pad_rows, 99.0, dtype=np.float32)
        rel[0:n] = (batch[lo:hi] - SEG_PER_CORE * j).astype(np.float32)
        assert n == 0 or (rel[0:n].min() >= 0 and rel[0:n].max() < W_WIN)

        cols = []
        for row0, nrows in tiles:
            cols.append(rel[row0:row0 + nrows].reshape(128, nrows // 128))
        bp = np.concatenate(cols, axis=1)

        in_maps.append({
            "x": xj,
            "batchp": np.ascontiguousarray(bp),
            "wt": wt,
            "bb": bb,
        })
    return in_maps


_NC_CACHE = {}


def kernel(x, W, b, batch, num_segments, trace=False):
    assert int(num_segments) == N_SEG
    batch_np = np.asarray(batch).astype(np.int64)
    # shard at segment boundaries: core j owns segments [32j, 32j+32)
    bnd = np.searchsorted(batch_np, np.arange(0, N_SEG + 1, SEG_PER_CORE))
    pad_rows = int(-(-int(np.diff(bnd).max()) // 256) * 256)
    if pad_rows not in _NC_CACHE:
        _NC_CACHE[pad_rows] = build_nc(pad_rows)
    nc = _NC_CACHE[pad_rows]
    in_maps = make_in_maps(x, W, b, batch, pad_rows, bnd)
    res = run_bass_kernel_spmd(nc, in_maps, core_ids=list(range(N_CORES)),
                               trace=trace)
    full = np.concatenate([res.results[j]["out"] for j in range(N_CORES)],
                          axis=0)
    if trace:
        return full, res
    return full
